# revision 38
# baseline (speedup 1.0000x reference)
"""Multi-head attention ('general' similarity, softmax, out-proj) on 8 trn2
NeuronCores via Bass/Tile.

Sharding: core c handles batch b=c//2, query rows [qh*1024, (qh+1)*1024) with
qh=c%2. Each core computes its own projections (full K/V for its batch), all 8
heads of attention for its query slice, and its slice of the output
projection. Outputs are disjoint -> host just concatenates.

Device layout trick: everything is kept feature-major ("transposed") so every
matmul contraction runs along SBUF partitions:
  Q_l^T[e,q]   = sum_d W_Q[d,e] Q^T[d,q]          (host supplies Q^T)
  K_hw^T[e,s]  = sum_d Wg[d,e] K_l^T[d,s]         (per head, d=e=64)
  scores^T[k,q]= sum_e K_hw^T[e,k] Q_l^T[e,q]     (row-packed: 2 heads share PE)
  P^T          = exp(scores^T)                     (ScalarE, psum->sbuf bf16)
  headaug^T    = sum_k Vaug[k,(v,1)] P^T[k,q]     (65th 'ones' col => rowsums free)
  out[q,o]     = sum_hv (head^T/rowsum)[hv,q] W_m[hv,o]
"""
import sys
import types

import numpy as np
import ml_dtypes

# ---------------------------------------------------------------- axon shim --
# antenv in this image lacks axon_hooks; register the NTFF profiling hook
# ourselves so trace=True works when the caller asks for it.
def _ensure_axon_hooks():
    if 'antenv.axon_hooks' in sys.modules:
        return
    try:
        from trn_agent_boot.trn_boot import _ntff_profile_via_ctypes
        hook = _ntff_profile_via_ctypes('/opt/axon/libaxon_pjrt.so')
    except Exception:
        hook = None
    mod = types.ModuleType('antenv.axon_hooks')
    mod.get_axon_ntff_profile_hook = lambda: hook
    mod.set_axon_ntff_profile_hook = lambda h: None
    sys.modules['antenv.axon_hooks'] = mod


_ensure_axon_hooks()

import concourse.bass as bass
import concourse.mybir as mybir
import concourse.tile as tile
from concourse.bass_utils import run_bass_kernel_spmd

BF16 = mybir.dt.bfloat16
F32 = mybir.dt.float32

P = 128
D = 512          # model dim (= D_K = D_V = D_OUT)
SQ = 1024        # query rows per core
SK = 2048        # key rows (full sequence)
H = 8
DH = 64
NJ = H // 2      # head pairs
NKB = SK // P    # 16 key blocks
NQB = SQ // P    # 8 query blocks
ND = D // P      # 4 feature blocks
EXPF = mybir.ActivationFunctionType.Exp
MULT = mybir.AluOpType.mult


# ------------------------------------------------------- walrus workaround --
# This container's walrus accepts only ONE embedded sync-wait per hw
# instruction. Move all but the last wait of any instruction onto single-wait
# NoOps inserted just before it in the same engine stream.
_SPLIT_CTR = [0]


def _split_multi_waits(nc, max_waits=1):
    def mk_nop(engine, wait):
        _SPLIT_CTR[0] += 1
        nop = mybir.InstNoOp(name=f"antsplitw-{_SPLIT_CTR[0]}", ins=[], outs=[])
        nop.engine = engine
        nop.sync_info = mybir.SyncInfo(on_wait=[wait], on_update=[])
        return nop

    for fn in nc.m.functions:
        for bb in fn.blocks:
            out = []
            changed = False
            for inst in bb.instructions:
                si = inst.sync_info
                waits = list(si.on_wait) if si is not None and si.on_wait else []
                if len(waits) > max_waits:
                    for w in waits[:-max_waits]:
                        out.append(mk_nop(inst.engine, w))
                    si.on_wait = waits[-max_waits:]
                    changed = True
                out.append(inst)
            if changed:
                bb.instructions = out


# ------------------------------------------------------------ device kernel --
def _build_nc():
    nc = bass.Bass("TRN2", target_bir_lowering=False, debug=False)

    qt_d = nc.declare_dram_parameter("qt", [D, SQ], BF16, isOutput=False)
    kt_d = nc.declare_dram_parameter("kt", [D, SK], BF16, isOutput=False)
    vt_d = nc.declare_dram_parameter("vt", [D, SK], BF16, isOutput=False)
    wq_d = nc.declare_dram_parameter("wq", [D, D], BF16, isOutput=False)
    # wkg = W_K @ blockdiag(W_gen_S) folded on the host: K_hw^T comes
    # straight out of the K projection
    wkg_d = nc.declare_dram_parameter("wkg", [D, D], BF16, isOutput=False)
    wv_d = nc.declare_dram_parameter("wv", [D, D], BF16, isOutput=False)
    wm_d = nc.declare_dram_parameter("wm", [D, D], BF16, isOutput=False)
    out_d = nc.declare_dram_parameter("out", [SQ, D], F32, isOutput=True)

    with tile.TileContext(nc) as tc:
        with tc.tile_pool(name="cst", bufs=1) as cst, \
             tc.tile_pool(name="pt", bufs=16) as ptp, \
             tc.tile_pool(name="dve", bufs=2) as dvp, \
             tc.tile_pool(name="psS", bufs=2, space="PSUM") as psS, \
             tc.tile_pool(name="psV", bufs=2, space="PSUM") as psV:

            # ---- loads (128-partition layout), ordered so pair 0's
            # operands land first
            wq = cst.tile([P, ND, D], BF16, tag="wq")
            nc.sync.dma_start(wq[:], wq_d.rearrange("(k p) e -> p k e", p=P))
            qt = cst.tile([P, ND, SQ], BF16, tag="qt")
            nc.sync.dma_start(qt[:], qt_d.rearrange("(k p) q -> p k q", p=P))
            wkg = cst.tile([P, ND, D], BF16, tag="wkg")
            nc.sync.dma_start(wkg[:], wkg_d.rearrange("(k p) e -> p k e", p=P))
            kt = cst.tile([P, ND, SK], BF16, tag="kt")
            kt_r = kt_d.rearrange("(k p) s -> p k s", p=P)
            nc.sync.dma_start(kt[:, :, 0:SK // 2], kt_r[:, :, 0:SK // 2])
            nc.sync.dma_start(kt[:, :, SK // 2:], kt_r[:, :, SK // 2:])
            wv = cst.tile([P, ND, D], BF16, tag="wv")
            nc.sync.dma_start(wv[:], wv_d.rearrange("(k p) e -> p k e", p=P))
            vt = cst.tile([P, ND, SK], BF16, tag="vt")
            vt_r = vt_d.rearrange("(k p) s -> p k s", p=P)
            nc.sync.dma_start(vt[:, :, 0:SK // 2], vt_r[:, :, 0:SK // 2])
            nc.sync.dma_start(vt[:, :, SK // 2:], vt_r[:, :, SK // 2:])
            wm = cst.tile([P, ND, D], BF16, tag="wm")
            nc.sync.dma_start(wm[:], wm_d.rearrange("(k p) e -> p k e", p=P))

            # fp32 selector for the rowsum-reciprocal partition broadcast:
            # row 0 -> output partitions 0:64, row 32 -> partitions 64:128
            sel = cst.tile([DH, P], BF16, tag="sel")
            nc.vector.memset(sel[:], 0.0)
            nc.vector.memset(sel[0:1, 0:DH], 1.0)
            nc.vector.memset(sel[32:33, DH:P], 1.0)

            # ---- projections (emitted as chunks, interleaved below) ----
            qlt = [cst.tile([P, SQ], BF16, tag=f"qlt{j}", name=f"qlt{j}") for j in range(NJ)]
            vaug = [cst.tile([P, H, DH + 1], BF16, tag=f"vaug{i}", name=f"vaug{i}")
                    for i in range(NKB)]
            khwt = [cst.tile([P, SK], BF16, tag=f"khwt{j}", name=f"khwt{j}") for j in range(NJ)]
            # partial output projection (head pairs 0-2), finished in the tail
            pout = [cst.tile([P, D], F32, tag=f"pout{qb}", name=f"pout{qb}")
                    for qb in range(NQB)]

            def qproj(j):
                # Q_l^T tile j (rows = e in [128j, 128j+128))
                ps = psS.tile([P, 1024], F32, tag="psS", name="psq")
                for qc in range(2):
                    for k in range(ND):
                        nc.tensor.matmul(
                            ps[:, qc * 512:(qc + 1) * 512],
                            wq[:, k, j * P:(j + 1) * P],
                            qt[:, k, qc * 512:(qc + 1) * 512],
                            start=(k == 0), stop=(k == ND - 1))
                nc.vector.tensor_copy(out=qlt[j][:], in_=ps[:])

            def kproj(j, half):
                # directly produces K_hw^T (W_gen_S folded into the weights):
                # rows 0:64 = head 2j, rows 64:128 = head 2j+1
                ps = psS.tile([P, 1024], F32, tag="psS", name="psk")
                for qc in range(2):
                    sc = half * 1024 + qc * 512
                    for k in range(ND):
                        nc.tensor.matmul(
                            ps[:, qc * 512:(qc + 1) * 512],
                            wkg[:, k, j * P:(j + 1) * P],
                            kt[:, k, sc:sc + 512],
                            start=(k == 0), stop=(k == ND - 1))
                nc.vector.tensor_copy(
                    out=khwt[j][:, half * 1024:(half + 1) * 1024], in_=ps[:])

            def kproj_q(j, sc):
                # quarter-granular kproj so the first scores can start early
                ps = psS.tile([P, 1024], F32, tag="psS", name="pskq")
                for k in range(ND):
                    nc.tensor.matmul(
                        ps[:, 0:512],
                        wkg[:, k, j * P:(j + 1) * P],
                        kt[:, k, sc:sc + 512],
                        start=(k == 0), stop=(k == ND - 1))
                nc.vector.tensor_copy(
                    out=khwt[j][:, sc:sc + 512], in_=ps[:, 0:512])

            def vproj(i):
                # V_l rows [128i, 128i+128), all heads + the ones column
                ps = psS.tile([P, 1024], F32, tag="psS", name="psv")
                for k in range(ND):
                    nc.tensor.matmul(
                        ps[:, 0:512],
                        vt[:, k, i * P:(i + 1) * P],
                        wv[:, k, :],
                        start=(k == 0), stop=(k == ND - 1))
                nc.vector.tensor_copy(
                    out=vaug[i][:, :, 0:DH],
                    in_=ps[:, 0:512].rearrange("p (h v) -> p h v", v=DH))
                nc.vector.memset(vaug[i][:, :, DH:DH + 1], 1.0)

            def outproj_partial(qb, headt=None):
                ps = psS.tile([P, 1024], F32, tag="psS", name="pop")
                for j in range(NJ - 1):
                    nc.tensor.matmul(ps[:, 0:512],
                                     headt[j][:, qb * P:(qb + 1) * P],
                                     wm[:, j, :], start=(j == 0),
                                     stop=(j == NJ - 2))
                nc.vector.tensor_copy(out=pout[qb][:], in_=ps[:, 0:512])

            # ---- attention: one continuous software-pipelined stream over all
            # (pair, kblock) steps; PV trails scores by LAG so pair boundaries
            # never drain the pipe. Projections / normalize tails / partial
            # output projection slot into fixed stream positions.
            headt = [cst.tile([P, SQ], BF16, tag=f"headt{j}", name=f"headt{j}") for j in range(NJ)]
            LOGF = mybir.ActivationFunctionType.Ln
            LAG = 4
            SS = NJ * NKB
            pv_tiles = {}
            pts = {}

            def scores_step(pos):
                # per-head grouping: exp(A) only waits on A's two matmuls,
                # so ScalarE starts ~1 matmul earlier each step
                j, t = divmod(pos, NKB)
                psa = psS.tile([P, SQ], F32, tag="psS", name="psa")
                for qc in range(2):
                    s = qc * 512
                    nc.tensor.matmul(
                        psa[:, s:s + 512],
                        khwt[j][0:DH, t * P:(t + 1) * P],
                        qlt[j][0:DH, s:s + 512], start=True, stop=True)
                pta = ptp.tile([P, SQ], BF16, tag="pt", name="pta")
                nc.scalar.activation(pta[:], psa[:], EXPF)
                psb = psS.tile([P, SQ], F32, tag="psS", name="psb")
                for qc in range(2):
                    s = qc * 512
                    nc.tensor.matmul(
                        psb[:, s:s + 512],
                        khwt[j][DH:P, t * P:(t + 1) * P],
                        qlt[j][DH:P, s:s + 512], start=True, stop=True,
                        tile_position=(DH, 0))
                ptb = ptp.tile([P, SQ], BF16, tag="pt", name="ptb")
                nc.scalar.activation(ptb[:], psb[:], EXPF)
                pts[pos] = (pta, ptb)

            def pv_step(pos):
                j, t = divmod(pos, NKB)
                if t == 0:
                    pv_tiles[j] = (
                        psV.tile([DH + 1, SQ], F32, tag="psV", name="pva"),
                        psV.tile([DH + 1, SQ], F32, tag="psV", name="pvb"))
                pva, pvb = pv_tiles[j]
                pta, ptb = pts.pop(pos)
                st, sp = (t == 0), (t == NKB - 1)
                for qc in range(2):
                    s = qc * 512
                    nc.tensor.matmul(pva[:, s:s + 512],
                                     vaug[t][:, 2 * j, :],
                                     pta[:, s:s + 512], start=st, stop=sp)
                    nc.tensor.matmul(pvb[:, s:s + 512],
                                     vaug[t][:, 2 * j + 1, :],
                                     ptb[:, s:s + 512], start=st, stop=sp)

            def part1(j):
                # gather the two rowsum rows into 32-aligned partitions, then
                # 1/x = exp(-ln(x)) on ScalarE (fast, and off the PE path)
                pva, pvb = pv_tiles[j]
                recw = dvp.tile([DH, SQ], F32, tag="recw", name="recw")
                nc.gpsimd.memset(recw[:], 1.0)
                nc.vector.tensor_copy(out=recw[0:1, :], in_=pva[DH:DH + 1, :])
                nc.vector.tensor_copy(out=recw[32:33, :], in_=pvb[DH:DH + 1, :])
                lg = dvp.tile([DH, SQ], F32, tag="lg", name="lg")
                nc.scalar.activation(lg[:], recw[:], LOGF)
                recr = dvp.tile([DH, SQ], BF16, tag="recr", name="recr")
                nc.scalar.activation(recr[:], lg[:], EXPF, scale=-1.0)
                return recr

            recrs = {}

            def part2(j):
                pva, pvb = pv_tiles.pop(j)
                recr = recrs.pop(j)
                rbp = psS.tile([P, SQ], F32, tag="psS", name="rbp")
                for qc in range(2):
                    s = qc * 512
                    nc.tensor.matmul(rbp[:, s:s + 512], sel[:],
                                     recr[:, s:s + 512], start=True, stop=True)
                rbe = dvp.tile([DH, SQ], F32, tag="rbe", name="rbe")
                rbo = dvp.tile([DH, SQ], F32, tag="rbo", name="rbo")
                nc.vector.tensor_copy(out=rbe[:], in_=rbp[0:DH, :])
                nc.vector.tensor_copy(out=rbo[:], in_=rbp[DH:P, :])
                nc.vector.tensor_tensor(headt[j][0:DH, :], pva[0:DH, :],
                                        rbe[:], MULT)
                nc.vector.tensor_tensor(headt[j][DH:P, :], pvb[0:DH, :],
                                        rbo[:], MULT)

            # stream schedule: extra chunks keyed by scores position
            pre = {}

            def at(pos, f):
                pre.setdefault(pos, []).append(f)

            for i in range(NKB):
                at(i, lambda i=i: vproj(i))
            at(26, lambda: qproj(2))
            at(28, lambda: kproj(2, 0))
            at(30, lambda: kproj(2, 1))
            at(42, lambda: qproj(3))
            at(44, lambda: kproj(3, 0))
            at(46, lambda: kproj(3, 1))
            for j in range(NJ - 1):
                at(NKB * (j + 1) + LAG, lambda j=j: part2(j))
            for qb in range(NQB):
                at(55 + qb, lambda qb=qb: outproj_partial(qb, headt))

            # prologue: warm the PE clock-gate with dummy matmuls while the
            # first DMAs land, then emit what pair 0's scores need
            warm = cst.tile([P, 512], BF16, tag="warm")
            nc.vector.memset(warm[:], 0.0)
            for _ in range(3):
                wps = psS.tile([P, 1024], F32, tag="psS", name="wps")
                for r in range(12):
                    nc.tensor.matmul(wps[:, (r % 2) * 512:(r % 2) * 512 + 512],
                                     warm[:, 0:P], warm[:],
                                     start=True, stop=True)
            qproj(0)
            kproj(0, 0)
            kproj(0, 1)
            qproj(1)
            kproj(1, 0)
            kproj(1, 1)

            pv_at = {}
            for p_ in range(SS):
                lag = LAG if p_ < 50 else 2
                pv_at.setdefault(p_ + lag, []).append(p_)
            for pos in range(SS + LAG):
                if pos < SS:
                    scores_step(pos)
                for f in pre.get(pos, []):
                    f()
                for p_ in pv_at.get(pos, []):
                    pv_step(p_)
                    pj, pt_ = divmod(p_, NKB)
                    if pt_ == NKB - 1:
                        recrs[pj] = part1(pj)

            # ---- tail: last pair's normalize + output projection, processed
            # per q-half so the two halves pipeline across PE/DVE/DMA
            jl = NJ - 1
            pvs_a, pvs_b = pv_tiles.pop(jl)
            recr = recrs.pop(jl)
            for qhalf in range(2):
                s = qhalf * 512
                rbp = psS.tile([P, 1024], F32, tag="psS", name="rbp")
                nc.tensor.matmul(rbp[:, 0:512], sel[:], recr[:, s:s + 512],
                                 start=True, stop=True)
                rbe = dvp.tile([DH, 512], F32, tag="rbeh", name="rbeh")
                rbo = dvp.tile([DH, 512], F32, tag="rboh", name="rboh")
                nc.vector.tensor_copy(out=rbe[:], in_=rbp[0:DH, 0:512])
                nc.vector.tensor_copy(out=rbo[:], in_=rbp[DH:P, 0:512])
                nc.vector.tensor_tensor(headt[jl][0:DH, s:s + 512],
                                        pvs_a[0:DH, s:s + 512], rbe[:], MULT)
                nc.vector.tensor_tensor(headt[jl][DH:P, s:s + 512],
                                        pvs_b[0:DH, s:s + 512], rbo[:], MULT)
                for qb in range(qhalf * 4, qhalf * 4 + 4):
                    ps = psS.tile([P, 1024], F32, tag="psS")
                    nc.tensor.matmul(ps[:, 0:512],
                                     headt[jl][:, qb * P:(qb + 1) * P],
                                     wm[:, jl, :], start=True, stop=True)
                    ot = dvp.tile([P, D], F32, tag="ot")
                    nc.vector.tensor_tensor(ot[:], ps[:, 0:512], pout[qb][:],
                                            mybir.AluOpType.add)
                    nc.sync.dma_start(out_d[qb * P:(qb + 1) * P, :], ot[:])

    _split_multi_waits(nc)
    return nc


_NC = None


def _get_nc():
    global _NC
    if _NC is None:
        _NC = _build_nc()
    return _NC


def _prep_in_maps(Q, K, V, W_Q, W_K, W_V, W_gen_S, W_multi_head):
    bf = ml_dtypes.bfloat16
    wq = np.ascontiguousarray(np.asarray(W_Q, np.float32)).astype(bf)
    wv = np.ascontiguousarray(np.asarray(W_V, np.float32)).astype(bf)
    wm = np.ascontiguousarray(np.asarray(W_multi_head, np.float32)).astype(bf)
    # fold W_gen_S into W_K: K_hw = K @ W_K @ blockdiag(W_gen_S)
    wk_f = np.asarray(W_K, np.float32)
    wg_f = np.asarray(W_gen_S, np.float32)
    wkg = np.einsum('dhe,ef->dhf', wk_f.reshape(D, H, DH), wg_f)
    wkg = np.ascontiguousarray(wkg.reshape(D, D)).astype(bf)

    Q = np.asarray(Q, np.float32)
    K = np.asarray(K, np.float32)
    V = np.asarray(V, np.float32)

    in_maps = []
    for c in range(8):
        b, qh = divmod(c, 2)
        qt = np.ascontiguousarray(
            Q[b, qh * SQ:(qh + 1) * SQ, :].T).astype(bf)
        kt = np.ascontiguousarray(K[b].T).astype(bf)
        vt = np.ascontiguousarray(V[b].T).astype(bf)
        in_maps.append({"qt": qt, "kt": kt, "vt": vt, "wq": wq, "wkg": wkg,
                        "wv": wv, "wm": wm})
    return in_maps


def _run(in_maps, trace=False):
    nc = _get_nc()
    res = run_bass_kernel_spmd(nc, in_maps, list(range(8)), trace=trace)
    out = np.empty((4, SK, D), np.float32)
    for c in range(8):
        b, qh = divmod(c, 2)
        out[b, qh * SQ:(qh + 1) * SQ, :] = res.results[c]["out"]
    return out, res


def kernel(Q, K, V, M, W_Q, W_K, W_V, W_gen_S, W_multi_head):
    in_maps = _prep_in_maps(Q, K, V, W_Q, W_K, W_V, W_gen_S, W_multi_head)
    out, _ = _run(in_maps, trace=False)
    return out


def kernel_traced(Q, K, V, M, W_Q, W_K, W_V, W_gen_S, W_multi_head):
    in_maps = _prep_in_maps(Q, K, V, W_Q, W_K, W_V, W_gen_S, W_multi_head)
    return _run(in_maps, trace=True)
